# revision 29
# baseline (speedup 1.0000x reference)
"""Bass/Tile TRN2 kernel for an (intentionally quirky) nn.MultiHeadAttention.

Problem shapes: B=8, S=256, D=4096, H=16, HD=256.
Sharding: pure data-parallel - one batch element per NeuronCore (8 cores).

Key algebraic fact exploited here: the module's mask zeroes head-dim
CHANNELS (same ~50% subset in every head) of both Q and K, so those
channels contribute nothing to the scores.  We therefore pack only the
surviving channels of Wq/Wk (per core, per its batch's mask), padded to
a common tile count across cores:  per head, `amain` full 128-row tiles
plus `rem` 32-row overflow units (32-row alignment keeps matmul
partition offsets legal).  This cuts both Q/K projection FLOPs and
Wq/Wk HBM traffic nearly in half.

Math (per batch b, surviving channel set C = {c : mask[b, c mod 256]!=0}):
    Q~ = (x_q @ Wq.T[:, C]) / 16       (1/sqrt(HD) folded into Wq)
    K~ = x_k @ Wk.T[:, C]
    per head h: S_T[t, s] = sum_{packed c} K~[t,c] Q~[s,c]
                P = exp(S_T)            (no max-sub needed; |scores| <~ 6)
                r[s] = 1 / sum_t P[t,s] (ones-matmul over partitions)
                A_h[c, s] = (sum_t V[t,c] P[t,s]) * r[s]
    V = x_v @ Wv.T;  out = attn @ (Wo/64).T   (natural [s, dout] layout)

All matmuls bf16 with fp32 PSUM accumulation (512-wide moving limit =
one PSUM bank per matmul).  V and out projections keep x (resp.
attn^T) stationary and stream 2048-wide weight DMA chunks.  The
softmax reciprocals are broadcast across partitions with a rank-1
ones-matmul on the PE (fp32) instead of a DRAM round-trip, and the
out projection is stored in natural [s, dout] layout as bf16.
"""

import os
import sys
import types
from itertools import cycle

sys.path.insert(0, "/opt/trn_rl_repo")

import numpy as np
import ml_dtypes

import concourse.bass as bass
import concourse.mybir as mybir
import concourse.tile as tile
from concourse.vector_clock import ScopedClock

BF16 = mybir.dt.bfloat16
F32 = mybir.dt.float32
NPBF16 = ml_dtypes.bfloat16

B, S, D, H = 8, 256, 4096, 16
HD = D // H          # 256
NK = D // 128        # 32 k-tiles of 128
N_CORES = 8

_drain_patched = False


def _patch_tile_drain():
    """This container's walrus build accepts only one sync-wait per
    instruction; Tile's exit drain collects one wait per logical processor.
    Split the waits across a chain of drains."""
    global _drain_patched
    if _drain_patched:
        return
    _drain_patched = True

    def patched(self, tick_clock, wait_clock):
        drain_inst = self.nc.sync.drain()
        inst = drain_inst.ins
        wait_clock.add_sem_waits(inst, ScopedClock({None: tick_clock.global_clock}))
        si = inst.sync_info
        if si is not None and len(si.on_wait) > 1:
            waits = list(si.on_wait)
            inst.sync_info = mybir.SyncInfo(
                on_wait=waits[:1], on_update=list(si.on_update)
            )
            for i in range(1, len(waits)):
                extra = self.nc.sync.drain()
                extra.ins.sync_info = mybir.SyncInfo(
                    on_wait=waits[i : i + 1], on_update=[]
                )
        self.nc.all_engine_barrier()
        popped = self.nc._tile_sem_poison_stack.pop()
        assert popped is self._sem_poison
        self.nc.clear_and_free_semaphores(list(self.sems.allocated().values()))
        self.nc.all_engine_barrier()

    tile.TileContext._drain_and_barrier = patched


_bir_patched = False


def _patch_bir_wait_split():
    """This walrus build accepts a single sync-wait per instruction.
    Tile's wait-assignment emits up to 4.  Split them in the serialized
    BIR: extra waits are carried by no-op RegisterMove instructions
    (imm 0 -> {Engine}_zero) inserted just before the overloaded
    instruction on the same engine.  Monotonic sem-ge waits make
    sequential waiting equivalent to simultaneous waiting.

    (Note: fusing Ldweights into self-loading Matmults to enable
    walrus --enable-ldw-opt=true compiles but yields all-zero output
    on hardware, so it is not done here.)"""
    global _bir_patched
    if _bir_patched:
        return
    _bir_patched = True
    import json as _json

    import concourse.bass2jax as b2j
    import concourse.bass_utils as bu

    orig_compile = b2j.compile_bir_kernel

    def fuse_ldweights(m):
        for fn in m.get("functions", []):
            for blk in fn.get("blocks", []):
                insts = blk.get("instructions", [])
                out = []
                pending = None
                for inst in insts:
                    if (
                        inst.get("opcode") == "Ldweights"
                        and inst.get("engine") == "PE"
                    ):
                        assert pending is None, "two Ldweights in a row"
                        pending = inst
                        continue
                    if pending is not None:
                        assert (
                            inst.get("opcode") == "Matmult"
                            and inst.get("engine") == "PE"
                        ), f"Ldweights followed by {inst.get('opcode')}"
                        lsi = pending.get("sync_info") or {}
                        msi = inst.get("sync_info") or {}
                        inst["sync_info"] = {
                            "on_wait": (lsi.get("on_wait") or [])
                            + (msi.get("on_wait") or []),
                            "on_update": (lsi.get("on_update") or [])
                            + (msi.get("on_update") or []),
                        }
                        pending = None
                    out.append(inst)
                assert pending is None, "trailing Ldweights"
                blk["instructions"] = out

    def split_waits(bir_str):
        m = _json.loads(bir_str)
        changed = False
        for fn in m.get("functions", []):
            for blk in fn.get("blocks", []):
                insts = blk.get("instructions", [])
                out = []
                for inst in insts:
                    si = inst.get("sync_info") or {}
                    waits = si.get("on_wait") or []
                    if len(waits) > 1 and all(
                        w.get("wait_mode") == "sem-ge-imm" for w in waits
                    ):
                        changed = True
                        eng = inst["engine"]
                        for i, w in enumerate(waits[:-1]):
                            out.append(
                                {
                                    "debug": inst.get("debug", 0),
                                    "engine": eng,
                                    "ins": [
                                        {
                                            "dtype": "int32",
                                            "kind": "imm_value",
                                            "value": 0,
                                        }
                                    ],
                                    "name": f"{inst['name']}_w{i}",
                                    "opcode": "RegisterMove",
                                    "outs": [
                                        {
                                            "dtype": "int32",
                                            "kind": "register_access",
                                            "regref": f"{eng}_zero",
                                        }
                                    ],
                                    "sync_info": {
                                        "on_update": [],
                                        "on_wait": [w],
                                    },
                                }
                            )
                        inst["sync_info"] = {
                            "on_update": si.get("on_update") or [],
                            "on_wait": [waits[-1]],
                        }
                    out.append(inst)
                blk["instructions"] = out
        if not changed:
            return bir_str
        return _json.dumps(m).encode()

    def wrapped(ant_bir_str, *args, **kwargs):
        return orig_compile(split_waits(ant_bir_str), *args, **kwargs)

    b2j.compile_bir_kernel = wrapped


def _install_ntff_hook():
    """Recreate the missing antenv.axon_hooks glue so trace=True can profile."""
    if "antenv.axon_hooks" in sys.modules:
        return
    mod = types.ModuleType("antenv.axon_hooks")
    mod._hook = None
    mod.set_axon_ntff_profile_hook = lambda h: setattr(mod, "_hook", h)
    mod.get_axon_ntff_profile_hook = lambda: mod._hook
    sys.modules["antenv.axon_hooks"] = mod
    try:
        import antenv

        antenv.axon_hooks = mod
        if "/root/.axon_site" not in sys.path:
            sys.path.insert(0, "/root/.axon_site")
        from trn_agent_boot.trn_boot import _ntff_profile_via_ctypes

        mod._hook = _ntff_profile_via_ctypes("/opt/axon/libaxon_pjrt.so")
        import concourse.bass_utils as bu

        bu.upload_artifacts = lambda tmpdir: tmpdir
    except Exception:
        pass


def _qk_tiles(amain, rem):
    """Packed Q/K dout tile count: 16*amain main tiles + overflow tiles
    holding 3 32-row units each (partition offsets may only be 0/32/64),
    padded to an even count for 2-tile PSUM groups."""
    novt = -(-16 * rem // 3) if rem else 0
    ntq = 16 * amain + novt
    return ntq + (ntq % 2)


def build_nc(amain, rem, with_bias):
    _patch_tile_drain()
    ntq = _qk_tiles(amain, rem)
    ngq = ntq // 2               # PSUM groups of 2 tiles
    nc = bass.Bass()

    xq = nc.dram_tensor("xq", [128, NK, S], BF16, kind="ExternalInput")
    xk = nc.dram_tensor("xk", [128, NK, S], BF16, kind="ExternalInput")
    xv = nc.dram_tensor("xv", [128, NK, S], BF16, kind="ExternalInput")
    # packed Q/K weights: [group of 2 dout tiles][kt pair][128 k][512]
    wqp = nc.dram_tensor("wqp", [ngq, 16, 128, 512], BF16, kind="ExternalInput")
    wkp = nc.dram_tensor("wkp", [ngq, 16, 128, 512], BF16, kind="ExternalInput")
    # V / O weight slabs: [round of 2048 dout][kt][128 k][2048]
    wv = nc.dram_tensor("wv", [2, NK, 128, 2048], BF16, kind="ExternalInput")
    wo = nc.dram_tensor("wo", [2, NK, 128, 2048], BF16, kind="ExternalInput")
    if with_bias:
        bqp = nc.dram_tensor("bqp", [128, ntq], F32, kind="ExternalInput")
        bkp = nc.dram_tensor("bkp", [128, ntq], F32, kind="ExternalInput")
        bvv = nc.dram_tensor("bvv", [1, D], BF16, kind="ExternalInput")
        bov = nc.dram_tensor("bov", [1, D], BF16, kind="ExternalInput")
    out = nc.dram_tensor("out", [2, 2, 128, 2048], BF16, kind="ExternalOutput")

    Ident = mybir.ActivationFunctionType.Identity
    Exp = mybir.ActivationFunctionType.Exp

    with tile.TileContext(nc) as tc:
        from contextlib import ExitStack

        with ExitStack() as ctx:
            resid = ctx.enter_context(tc.tile_pool(name="resid", bufs=1))
            outp = ctx.enter_context(tc.tile_pool(name="outp", bufs=2))

            # ---- resident SBUF tensors ----
            qt_sb = resid.tile([128, ntq, S], BF16, tag="qt")   # packed Q^T
            kt_sb = resid.tile([128, ntq, S], BF16, tag="kt")   # packed K^T
            v0_sb = resid.tile([128, D], BF16, tag="v0")        # V[t=0:128, c]
            v1_sb = resid.tile([128, D], BF16, tag="v1")        # V[t=128:256, c]
            at_sb = resid.tile([128, NK, S], BF16, tag="at")    # attn^T [c, s]
            xv_sb = resid.tile([128, NK, S], BF16, tag="xv")
            et4 = [
                resid.tile([128, 8, S], BF16, name="et", tag=f"et{i}")
                for i in range(4)
            ]
            rinv_b = resid.tile([1, H, S], BF16, tag="rib")
            rbc = resid.tile([128, H, S], F32, tag="rbc")       # bcast recips
            ones_a = resid.tile([128, 128], BF16, tag="ones")
            ones_row = ones_a[0:1, :]    # [1, 128]
            ones_col = ones_a[:, 0:1]    # [128, 1]
            if with_bias:
                ball = resid.tile([128, 2, ntq], F32, tag="ball")
                bq_sb = ball[:, 0, :]
                bk_sb = ball[:, 1, :]
                bv_sb = resid.tile([1, D], BF16, tag="bv")
                bo_sb = resid.tile([1, D], BF16, tag="bo")

            nc.vector.memset(ones_a[:], 1.0)

            qk_queues = cycle([nc.scalar, nc.sync])
            queues = cycle([nc.scalar, nc.sync, nc.gpsimd])

            # ---- packed Q/K projections (weight-stationary, out = W^T x) ----
            def proj_packed(w_dram, x_sb, dst_sb, bias_sb, pspool, wpool):
                for g in range(ngq):
                    ps = [
                        pspool.tile([128, S], F32, name="ps", tag="ps")
                        for _ in range(2)
                    ]
                    for kp in range(16):
                        ch = wpool.tile([128, 512], BF16, name="cq", tag="cq")
                        next(qk_queues).dma_start(out=ch[:], in_=w_dram[g, kp])
                        for half in range(2):
                            kt = 2 * kp + half
                            for j in range(2):
                                nc.tensor.matmul(
                                    ps[j][:],
                                    lhsT=ch[:, half * 256 + j * 128 : half * 256 + (j + 1) * 128],
                                    rhs=x_sb[:, kt, :],
                                    start=(kt == 0),
                                    stop=(kt == NK - 1),
                                )
                    for j in range(2):
                        t = g * 2 + j
                        if with_bias:
                            nc.scalar.activation(
                                out=dst_sb[:, t, :], in_=ps[j][:], func=Ident,
                                bias=bias_sb[:, t : t + 1],
                            )
                        else:
                            nc.scalar.activation(
                                out=dst_sb[:, t, :], in_=ps[j][:], func=Ident
                            )

            with tc.tile_pool(name="xqk", bufs=1) as xqkp:
                xq_sb = xqkp.tile([128, NK, S], BF16, tag="xq")
                nc.sync.dma_start(out=xq_sb[:, 0:16, :], in_=xq[:, 0:16, :])
                nc.gpsimd.dma_start(out=xq_sb[:, 16:32, :], in_=xq[:, 16:32, :])
                xk_sb = xqkp.tile([128, NK, S], BF16, tag="xk")
                nc.gpsimd.dma_start(out=xk_sb[:], in_=xk[:])
                nc.gpsimd.dma_start(out=xv_sb[:], in_=xv[:])
                if with_bias:
                    nc.gpsimd.dma_start(out=ball[:, 0, :], in_=bqp[:])
                    nc.gpsimd.dma_start(out=ball[:, 1, :], in_=bkp[:])
                    nc.gpsimd.dma_start(out=bv_sb[:], in_=bvv[:])
                    nc.gpsimd.dma_start(out=bo_sb[:], in_=bov[:])
                with (
                    tc.tile_pool(name="wqk", bufs=20) as wqk,
                    tc.tile_pool(name="psqk", bufs=8, space="PSUM") as psqk,
                ):
                    proj_packed(wqp, xq_sb, qt_sb,
                                bq_sb if with_bias else None, psqk, wqk)
                    proj_packed(wkp, xk_sb, kt_sb,
                                bk_sb if with_bias else None, psqk, wqk)

            # ---- V projection (x stationary, 2048-wide weight stream) ----
            with tc.tile_pool(name="wvo", bufs=12) as wvo:
                with tc.tile_pool(name="psv", bufs=8, space="PSUM") as psv:
                    for r in range(2):
                        pv = [
                            [
                                psv.tile([128, 512], F32, name="pv", tag="pv")
                                for _ in range(4)
                            ]
                            for _ in range(2)
                        ]
                        if with_bias:
                            for tt in range(2):
                                for di in range(4):
                                    bsl = slice(r * 2048 + di * 512,
                                                r * 2048 + (di + 1) * 512)
                                    nc.tensor.matmul(
                                        pv[tt][di][:], lhsT=ones_row[:],
                                        rhs=bv_sb[:, bsl],
                                        start=True, stop=False,
                                    )
                        for kt in range(NK):
                            ch = wvo.tile([128, 2048], BF16, name="cv", tag="cvo")
                            next(queues).dma_start(out=ch[:], in_=wv[r, kt])
                            for tt in range(2):
                                for di in range(4):
                                    nc.tensor.matmul(
                                        pv[tt][di][:],
                                        lhsT=xv_sb[:, kt, tt * 128 : (tt + 1) * 128],
                                        rhs=ch[:, di * 512 : (di + 1) * 512],
                                        start=(kt == 0 and not with_bias),
                                        stop=(kt == NK - 1),
                                    )
                        for tt in range(2):
                            vdst = v0_sb if tt == 0 else v1_sb
                            for di in range(4):
                                dsl = slice(r * 2048 + di * 512,
                                            r * 2048 + (di + 1) * 512)
                                if tt == 0:
                                    nc.scalar.activation(
                                        out=vdst[:, dsl], in_=pv[tt][di][:],
                                        func=Ident,
                                    )
                                else:
                                    nc.vector.tensor_copy(
                                        vdst[:, dsl], pv[tt][di][:]
                                    )

                # ---- attention ----
                # phase A: per head scores^T, exp, col sums, reciprocals
                with (
                    tc.tile_pool(name="psa", bufs=4, space="PSUM") as psa,
                    tc.tile_pool(name="psr", bufs=2, space="PSUM") as psr,
                    tc.tile_pool(name="psb", bufs=1, space="PSUM") as psb,
                ):
                    novt = 16 * amain  # first overflow tile index
                    for h in range(H):
                        eth = et4[h // 4][:, (h % 4) * 2 : (h % 4) * 2 + 2, :]
                        nmm = amain + rem
                        for tt in range(2):
                            pss = psa.tile([128, S], F32, name="pa", tag="pa")
                            tsl = slice(tt * 128, (tt + 1) * 128)
                            i = 0
                            for t in range(amain):
                                mt = h * amain + t
                                nc.tensor.matmul(
                                    pss[:],
                                    lhsT=kt_sb[:, mt, tsl],
                                    rhs=qt_sb[:, mt, :],
                                    start=(i == 0), stop=(i == nmm - 1),
                                )
                                i += 1
                            for ov in range(rem):
                                u = h * rem + ov
                                ovt = novt + u // 3
                                ro = 32 * (u % 3)
                                nc.tensor.matmul(
                                    pss[:],
                                    lhsT=kt_sb[ro : ro + 32, ovt, tsl],
                                    rhs=qt_sb[ro : ro + 32, ovt, :],
                                    start=(i == 0), stop=(i == nmm - 1),
                                )
                                i += 1
                            nc.scalar.activation(
                                out=eth[:, tt, :], in_=pss[:], func=Exp
                            )
                        # column sums of exp (over t = partitions) via matmul
                        prs = psr.tile([1, S], F32, name="pr", tag="pr")
                        nc.tensor.matmul(
                            prs[:], lhsT=ones_col[:], rhs=eth[:, 0, :],
                            start=True, stop=False,
                        )
                        nc.tensor.matmul(
                            prs[:], lhsT=ones_col[:], rhs=eth[:, 1, :],
                            start=False, stop=True,
                        )
                        with nc.allow_low_precision(
                            reason="softmax reciprocal in bf16"
                        ):
                            nc.vector.reciprocal(rinv_b[:, h, :], prs[:])

                    # phase B: attention @ V; reciprocals are broadcast
                    # across partitions by a rank-1 ones-matmul per 4 heads
                    for q in range(4):
                        for hh in range(2):
                            pb = psb.tile([128, 2 * S], F32, name="pb", tag="pb")
                            h0 = 4 * q + 2 * hh
                            nc.tensor.matmul(
                                pb[:], lhsT=ones_row[:],
                                rhs=rinv_b[:, h0 : h0 + 2, :],
                                start=True, stop=True,
                            )
                            nc.vector.tensor_copy(rbc[:, h0 : h0 + 2, :], pb[:])
                        for h in range(4 * q, 4 * q + 4):
                            for j in range(2):
                                csl = slice(h * HD + j * 128, h * HD + (j + 1) * 128)
                                pu = psa.tile([128, S], F32, name="pc", tag="pa")
                                nc.tensor.matmul(
                                    pu[:], lhsT=v0_sb[:, csl],
                                    rhs=et4[h // 4][:, (h % 4) * 2, :],
                                    start=True, stop=False,
                                )
                                nc.tensor.matmul(
                                    pu[:], lhsT=v1_sb[:, csl],
                                    rhs=et4[h // 4][:, (h % 4) * 2 + 1, :],
                                    start=False, stop=True,
                                )
                                nc.vector.tensor_mul(
                                    at_sb[:, 2 * h + j, :], pu[:], rbc[:, h, :]
                                )

                # ---- output projection (attn^T stationary, Wo streamed) ----
                with tc.tile_pool(name="pso", bufs=8, space="PSUM") as pso:
                    for r in range(2):
                        po = [
                            [
                                pso.tile([128, 512], F32, name="po", tag="po")
                                for _ in range(4)
                            ]
                            for _ in range(2)
                        ]
                        if with_bias:
                            for sh in range(2):
                                for di in range(4):
                                    bsl = slice(r * 2048 + di * 512,
                                                r * 2048 + (di + 1) * 512)
                                    nc.tensor.matmul(
                                        po[sh][di][:], lhsT=ones_row[:],
                                        rhs=bo_sb[:, bsl],
                                        start=True, stop=False,
                                    )
                        for ct in range(NK):
                            ch = wvo.tile([128, 2048], BF16, name="co", tag="cvo")
                            next(queues).dma_start(out=ch[:], in_=wo[r, ct])
                            for sh in range(2):
                                for di in range(4):
                                    nc.tensor.matmul(
                                        po[sh][di][:],
                                        lhsT=at_sb[:, ct, sh * 128 : (sh + 1) * 128],
                                        rhs=ch[:, di * 512 : (di + 1) * 512],
                                        start=(ct == 0 and not with_bias),
                                        stop=(ct == NK - 1),
                                    )
                        for sh in range(2):
                            ot = outp.tile([128, 2048], BF16, name="ot", tag="ot")
                            for di in range(4):
                                osl = slice(di * 512, (di + 1) * 512)
                                if sh == 0:
                                    nc.scalar.activation(
                                        out=ot[:, osl], in_=po[sh][di][:],
                                        func=Ident,
                                    )
                                else:
                                    nc.vector.tensor_copy(
                                        ot[:, osl], po[sh][di][:]
                                    )
                            nc.sync.dma_start(out=out[sh, r], in_=ot[:])

    return nc


_cached = {}


def _get_nc(key=None):
    if key is None:
        key = _cached.get("last_key")
    assert key is not None, "build_in_maps must run before _get_nc"
    if key not in _cached:
        _cached[key] = build_nc(*key)
    _cached["last_key"] = key
    return _cached[key]


def _pack_gidx(mask_b, amain, rem, ntq):
    """Map packed column p -> global W column (h*256 + channel), -1 = pad."""
    surv = np.nonzero(np.asarray(mask_b) != 0)[0]
    M = len(surv)
    P = ntq * 128
    gidx = np.full(P, -1, np.int64)
    main_n = min(M, 128 * amain)
    for h in range(H):
        base = h * amain * 128
        gidx[base : base + main_n] = h * 256 + surv[:main_n]
        for i in range(rem):
            u = h * rem + i
            t = 16 * amain + u // 3
            ro = 32 * (u % 3)
            p0 = t * 128 + ro
            lo = 128 * amain + i * 32
            n = min(max(M - lo, 0), 32)
            if n > 0:
                gidx[p0 : p0 + n] = h * 256 + surv[lo : lo + n]
    return gidx


def _pack_w(W, scale, gidx, ngq):
    wt = np.ascontiguousarray(W.T * scale).astype(np.float32)  # [k, dout]
    P = len(gidx)
    pk = np.zeros((D, P), np.float32)
    valid = gidx >= 0
    pk[:, valid] = wt[:, gidx[valid]]
    pk = pk.astype(NPBF16)
    pk = pk.reshape(16, 2, 128, ngq, 256).transpose(3, 0, 2, 1, 4)
    return np.ascontiguousarray(pk.reshape(ngq, 16, 128, 512))


def _chunks2048(W, scale):
    # W.T with dout split into 2 rounds of 2048: [2, 32, 128, 2048]
    wt = (W.T * scale).astype(NPBF16)  # [k, dout]
    return np.ascontiguousarray(
        wt.reshape(NK, 128, 2, 2048).transpose(2, 0, 1, 3)
    )


def build_in_maps(q, k, v, mask, Wq, bq, Wk, bk, Wv, bv, Wo, bo):
    q = np.asarray(q, dtype=np.float32)
    k = np.asarray(k, dtype=np.float32)
    v = np.asarray(v, dtype=np.float32)
    mask = np.asarray(mask)
    Wq, bq = np.asarray(Wq, np.float32), np.asarray(bq, np.float32)
    Wk, bk = np.asarray(Wk, np.float32), np.asarray(bk, np.float32)
    Wv, bv = np.asarray(Wv, np.float32), np.asarray(bv, np.float32)
    Wo, bo = np.asarray(Wo, np.float32), np.asarray(bo, np.float32)

    Ms = [(mask[b] != 0).sum() for b in range(B)]
    Mhat = max(32, int(-(-max(Ms) // 32) * 32))  # ceil to 32, common
    amain, rem = Mhat // 128, (Mhat % 128) // 32
    ntq = _qk_tiles(amain, rem)
    ngq = ntq // 2
    with_bias = any(np.any(x != 0) for x in (bq, bk, bv, bo))
    key = (amain, rem, with_bias)
    _cached["last_key"] = key

    wv_c = _chunks2048(Wv, 1.0)
    wo_c = _chunks2048(Wo, 1.0 / 64.0)
    if with_bias:
        bvv = np.ascontiguousarray(bv.astype(NPBF16).reshape(1, D))
        bov = np.ascontiguousarray(bo.astype(NPBF16).reshape(1, D))

    in_maps = []
    for b in range(B):
        gidx = _pack_gidx(mask[b], amain, rem, ntq)
        wq_p = _pack_w(Wq, 1.0 / 16.0, gidx, ngq)
        wk_p = _pack_w(Wk, 1.0, gidx, ngq)

        def xt(x):
            # [128 partition, NK k-tile, S] with 16KB contiguous per partition
            t = x[b].T.reshape(NK, 128, S).swapaxes(0, 1)
            return np.ascontiguousarray(t).astype(NPBF16)

        im = dict(xq=xt(q), xk=xt(k), xv=xt(v), wqp=wq_p, wkp=wk_p,
                  wv=wv_c, wo=wo_c)
        if with_bias:
            def pb(bias):
                pkb = np.zeros(ntq * 128, np.float32)
                valid = gidx >= 0
                pkb[valid] = (bias / 16.0 if bias is bq else bias)[gidx[valid] ]
                return np.ascontiguousarray(pkb.reshape(ntq, 128).T)
            im.update(bqp=pb(bq), bkp=pb(bk), bvv=bvv, bov=bov)
        in_maps.append(im)
    return in_maps


def unshard(results):
    outs = []
    for b in range(B):
        blk = np.asarray(results[b]["out"], np.float32)  # [2, 2, 128, 2048]
        o = np.empty((S, D), np.float32)
        for sh in range(2):
            for r in range(2):
                o[sh * 128 : (sh + 1) * 128, r * 2048 : (r + 1) * 2048] = blk[sh, r]
        outs.append(o)
    return np.ascontiguousarray(np.stack(outs))


def kernel(q, k, v, mask, Wq, bq, Wk, bk, Wv, bv, Wo, bo):
    _install_ntff_hook()
    _patch_bir_wait_split()
    in_maps = build_in_maps(q, k, v, mask, Wq, bq, Wk, bk, Wv, bv, Wo, bo)
    nc = _get_nc()

    from concourse.bass_utils import run_bass_kernel_spmd

    res = run_bass_kernel_spmd(nc, in_maps, list(range(N_CORES)))
    return unshard(res.results)


# revision 30
# speedup vs baseline: 1.0330x; 1.0330x over previous
"""Bass/Tile TRN2 kernel for an (intentionally quirky) nn.MultiHeadAttention.

Problem shapes: B=8, S=256, D=4096, H=16, HD=256.
Sharding: pure data-parallel - one batch element per NeuronCore (8 cores).

Key algebraic fact exploited here: the module's mask zeroes head-dim
CHANNELS (same ~50% subset in every head) of both Q and K, so those
channels contribute nothing to the scores.  We therefore pack only the
surviving channels of Wq/Wk (per core, per its batch's mask), padded to
a common tile count across cores:  per head, `amain` full 128-row tiles
plus `rem` 32-row overflow units (32-row alignment keeps matmul
partition offsets legal).  This cuts both Q/K projection FLOPs and
Wq/Wk HBM traffic nearly in half.

Math (per batch b, surviving channel set C = {c : mask[b, c mod 256]!=0}):
    Q~ = (x_q @ Wq.T[:, C]) / 16       (1/sqrt(HD) folded into Wq)
    K~ = x_k @ Wk.T[:, C]
    per head h: S_T[t, s] = sum_{packed c} K~[t,c] Q~[s,c]
                P = exp(S_T)            (no max-sub needed; |scores| <~ 6)
                r[s] = 1 / sum_t P[t,s] (ones-matmul over partitions)
                A_h[c, s] = (sum_t V[t,c] P[t,s]) * r[s]
    V = x_v @ Wv.T;  out = attn @ (Wo/64).T   (natural [s, dout] layout)

All matmuls bf16 with fp32 PSUM accumulation (512-wide moving limit =
one PSUM bank per matmul).  V and out projections keep x (resp.
attn^T) stationary and stream 2048-wide weight DMA chunks.  The
softmax reciprocals are broadcast across partitions with a rank-1
ones-matmul on the PE (fp32) instead of a DRAM round-trip, and the
out projection is stored in natural [s, dout] layout as bf16.
"""

import os
import sys
import types
from itertools import cycle

sys.path.insert(0, "/opt/trn_rl_repo")

import numpy as np
import ml_dtypes

import concourse.bass as bass
import concourse.mybir as mybir
import concourse.tile as tile
from concourse.vector_clock import ScopedClock

BF16 = mybir.dt.bfloat16
F32 = mybir.dt.float32
NPBF16 = ml_dtypes.bfloat16

B, S, D, H = 8, 256, 4096, 16
HD = D // H          # 256
NK = D // 128        # 32 k-tiles of 128
N_CORES = 8

_drain_patched = False


def _patch_tile_drain():
    """This container's walrus build accepts only one sync-wait per
    instruction; Tile's exit drain collects one wait per logical processor.
    Split the waits across a chain of drains."""
    global _drain_patched
    if _drain_patched:
        return
    _drain_patched = True

    def patched(self, tick_clock, wait_clock):
        drain_inst = self.nc.sync.drain()
        inst = drain_inst.ins
        wait_clock.add_sem_waits(inst, ScopedClock({None: tick_clock.global_clock}))
        si = inst.sync_info
        if si is not None and len(si.on_wait) > 1:
            waits = list(si.on_wait)
            inst.sync_info = mybir.SyncInfo(
                on_wait=waits[:1], on_update=list(si.on_update)
            )
            for i in range(1, len(waits)):
                extra = self.nc.sync.drain()
                extra.ins.sync_info = mybir.SyncInfo(
                    on_wait=waits[i : i + 1], on_update=[]
                )
        self.nc.all_engine_barrier()
        popped = self.nc._tile_sem_poison_stack.pop()
        assert popped is self._sem_poison
        self.nc.clear_and_free_semaphores(list(self.sems.allocated().values()))
        self.nc.all_engine_barrier()

    tile.TileContext._drain_and_barrier = patched


_bir_patched = False


def _patch_bir_wait_split():
    """This walrus build accepts a single sync-wait per instruction.
    Tile's wait-assignment emits up to 4.  Split them in the serialized
    BIR: extra waits are carried by no-op RegisterMove instructions
    (imm 0 -> {Engine}_zero) inserted just before the overloaded
    instruction on the same engine.  Monotonic sem-ge waits make
    sequential waiting equivalent to simultaneous waiting.

    (Note: fusing Ldweights into self-loading Matmults to enable
    walrus --enable-ldw-opt=true compiles but yields all-zero output
    on hardware, so it is not done here.)"""
    global _bir_patched
    if _bir_patched:
        return
    _bir_patched = True
    import json as _json

    import concourse.bass2jax as b2j
    import concourse.bass_utils as bu

    orig_compile = b2j.compile_bir_kernel

    def fuse_ldweights(m):
        for fn in m.get("functions", []):
            for blk in fn.get("blocks", []):
                insts = blk.get("instructions", [])
                out = []
                pending = None
                for inst in insts:
                    if (
                        inst.get("opcode") == "Ldweights"
                        and inst.get("engine") == "PE"
                    ):
                        assert pending is None, "two Ldweights in a row"
                        pending = inst
                        continue
                    if pending is not None:
                        assert (
                            inst.get("opcode") == "Matmult"
                            and inst.get("engine") == "PE"
                        ), f"Ldweights followed by {inst.get('opcode')}"
                        lsi = pending.get("sync_info") or {}
                        msi = inst.get("sync_info") or {}
                        inst["sync_info"] = {
                            "on_wait": (lsi.get("on_wait") or [])
                            + (msi.get("on_wait") or []),
                            "on_update": (lsi.get("on_update") or [])
                            + (msi.get("on_update") or []),
                        }
                        pending = None
                    out.append(inst)
                assert pending is None, "trailing Ldweights"
                blk["instructions"] = out

    def split_waits(bir_str):
        m = _json.loads(bir_str)
        changed = False
        for fn in m.get("functions", []):
            for blk in fn.get("blocks", []):
                insts = blk.get("instructions", [])
                out = []
                for inst in insts:
                    si = inst.get("sync_info") or {}
                    waits = si.get("on_wait") or []
                    if len(waits) > 1 and all(
                        w.get("wait_mode") == "sem-ge-imm" for w in waits
                    ):
                        changed = True
                        eng = inst["engine"]
                        for i, w in enumerate(waits[:-1]):
                            out.append(
                                {
                                    "debug": inst.get("debug", 0),
                                    "engine": eng,
                                    "ins": [
                                        {
                                            "dtype": "int32",
                                            "kind": "imm_value",
                                            "value": 0,
                                        }
                                    ],
                                    "name": f"{inst['name']}_w{i}",
                                    "opcode": "RegisterMove",
                                    "outs": [
                                        {
                                            "dtype": "int32",
                                            "kind": "register_access",
                                            "regref": f"{eng}_zero",
                                        }
                                    ],
                                    "sync_info": {
                                        "on_update": [],
                                        "on_wait": [w],
                                    },
                                }
                            )
                        inst["sync_info"] = {
                            "on_update": si.get("on_update") or [],
                            "on_wait": [waits[-1]],
                        }
                    out.append(inst)
                blk["instructions"] = out
        if not changed:
            return bir_str
        return _json.dumps(m).encode()

    def wrapped(ant_bir_str, *args, **kwargs):
        return orig_compile(split_waits(ant_bir_str), *args, **kwargs)

    b2j.compile_bir_kernel = wrapped


def _install_ntff_hook():
    """Recreate the missing antenv.axon_hooks glue so trace=True can profile."""
    if "antenv.axon_hooks" in sys.modules:
        return
    mod = types.ModuleType("antenv.axon_hooks")
    mod._hook = None
    mod.set_axon_ntff_profile_hook = lambda h: setattr(mod, "_hook", h)
    mod.get_axon_ntff_profile_hook = lambda: mod._hook
    sys.modules["antenv.axon_hooks"] = mod
    try:
        import antenv

        antenv.axon_hooks = mod
        if "/root/.axon_site" not in sys.path:
            sys.path.insert(0, "/root/.axon_site")
        from trn_agent_boot.trn_boot import _ntff_profile_via_ctypes

        mod._hook = _ntff_profile_via_ctypes("/opt/axon/libaxon_pjrt.so")
        import concourse.bass_utils as bu

        bu.upload_artifacts = lambda tmpdir: tmpdir
    except Exception:
        pass


def _qk_tiles(amain, rem):
    """Packed Q/K dout tile count: 16*amain main tiles + overflow tiles
    holding 3 32-row units each (partition offsets may only be 0/32/64),
    padded to an even count for 2-tile PSUM groups."""
    novt = -(-16 * rem // 3) if rem else 0
    ntq = 16 * amain + novt
    return ntq + (ntq % 2)


def build_nc(amain, rem, with_bias):
    _patch_tile_drain()
    ntq = _qk_tiles(amain, rem)
    ngq = ntq // 2               # PSUM groups of 2 tiles
    nc = bass.Bass()

    xq = nc.dram_tensor("xq", [128, NK, S], BF16, kind="ExternalInput")
    xk = nc.dram_tensor("xk", [128, NK, S], BF16, kind="ExternalInput")
    xv = nc.dram_tensor("xv", [128, NK, S], BF16, kind="ExternalInput")
    # packed Q/K weights: [group of 2 dout tiles][kt pair][128 k][512]
    wqp = nc.dram_tensor("wqp", [ngq, 16, 128, 512], BF16, kind="ExternalInput")
    wkp = nc.dram_tensor("wkp", [ngq, 16, 128, 512], BF16, kind="ExternalInput")
    # V / O weight slabs: [round of 2048 dout][kt][128 k][2048]
    wv = nc.dram_tensor("wv", [2, NK, 128, 2048], BF16, kind="ExternalInput")
    wo = nc.dram_tensor("wo", [2, NK, 128, 2048], BF16, kind="ExternalInput")
    if with_bias:
        bqp = nc.dram_tensor("bqp", [128, ntq], F32, kind="ExternalInput")
        bkp = nc.dram_tensor("bkp", [128, ntq], F32, kind="ExternalInput")
        bvv = nc.dram_tensor("bvv", [1, D], BF16, kind="ExternalInput")
        bov = nc.dram_tensor("bov", [1, D], BF16, kind="ExternalInput")
    out = nc.dram_tensor("out", [2, 2, 128, 2048], BF16, kind="ExternalOutput")

    Ident = mybir.ActivationFunctionType.Identity
    Exp = mybir.ActivationFunctionType.Exp

    with tile.TileContext(nc) as tc:
        from contextlib import ExitStack

        with ExitStack() as ctx:
            resid = ctx.enter_context(tc.tile_pool(name="resid", bufs=1))
            outp = ctx.enter_context(tc.tile_pool(name="outp", bufs=2))

            # ---- resident SBUF tensors ----
            qt_sb = resid.tile([128, ntq, S], BF16, tag="qt")   # packed Q^T
            kt_sb = resid.tile([128, ntq, S], BF16, tag="kt")   # packed K^T
            v0_sb = resid.tile([128, D], BF16, tag="v0")        # V[t=0:128, c]
            v1_sb = resid.tile([128, D], BF16, tag="v1")        # V[t=128:256, c]
            at_sb = resid.tile([128, NK, S], BF16, tag="at")    # attn^T [c, s]
            xv_sb = resid.tile([128, NK, S], BF16, tag="xv")
            et4 = [
                resid.tile([128, 8, S], BF16, name="et", tag=f"et{i}")
                for i in range(4)
            ]
            rinv_f = resid.tile([1, H, S], F32, tag="rif")
            rinv_b = resid.tile([1, H, S], BF16, tag="rib")
            rbc = resid.tile([128, H, S], F32, tag="rbc")       # bcast recips
            ones_a = resid.tile([128, 128], BF16, tag="ones")
            ones_row = ones_a[0:1, :]    # [1, 128]
            ones_col = ones_a[:, 0:1]    # [128, 1]
            if with_bias:
                ball = resid.tile([128, 2, ntq], F32, tag="ball")
                bq_sb = ball[:, 0, :]
                bk_sb = ball[:, 1, :]
                bv_sb = resid.tile([1, D], BF16, tag="bv")
                bo_sb = resid.tile([1, D], BF16, tag="bo")

            nc.vector.memset(ones_a[:], 1.0)

            queues = cycle([nc.scalar, nc.sync, nc.gpsimd])

            # ---- packed Q/K projections (weight-stationary, out = W^T x) ----
            def proj_packed(w_dram, x_sb, dst_sb, bias_sb, pspool, wpool):
                for g in range(ngq):
                    ps = [
                        pspool.tile([128, S], F32, name="ps", tag="ps")
                        for _ in range(2)
                    ]
                    for kp in range(16):
                        ch = wpool.tile([128, 512], BF16, name="cq", tag="cq")
                        next(queues).dma_start(out=ch[:], in_=w_dram[g, kp])
                        for half in range(2):
                            kt = 2 * kp + half
                            for j in range(2):
                                nc.tensor.matmul(
                                    ps[j][:],
                                    lhsT=ch[:, half * 256 + j * 128 : half * 256 + (j + 1) * 128],
                                    rhs=x_sb[:, kt, :],
                                    start=(kt == 0),
                                    stop=(kt == NK - 1),
                                )
                    for j in range(2):
                        t = g * 2 + j
                        if with_bias:
                            nc.scalar.activation(
                                out=dst_sb[:, t, :], in_=ps[j][:], func=Ident,
                                bias=bias_sb[:, t : t + 1],
                            )
                        else:
                            nc.scalar.activation(
                                out=dst_sb[:, t, :], in_=ps[j][:], func=Ident
                            )

            with tc.tile_pool(name="xqk", bufs=1) as xqkp:
                xq_sb = xqkp.tile([128, NK, S], BF16, tag="xq")
                nc.sync.dma_start(out=xq_sb[:], in_=xq[:])
                xk_sb = xqkp.tile([128, NK, S], BF16, tag="xk")
                nc.gpsimd.dma_start(out=xk_sb[:], in_=xk[:])
                nc.scalar.dma_start(out=xv_sb[:], in_=xv[:])
                if with_bias:
                    nc.gpsimd.dma_start(out=ball[:, 0, :], in_=bqp[:])
                    nc.gpsimd.dma_start(out=ball[:, 1, :], in_=bkp[:])
                    nc.gpsimd.dma_start(out=bv_sb[:], in_=bvv[:])
                    nc.gpsimd.dma_start(out=bo_sb[:], in_=bov[:])
                with (
                    tc.tile_pool(name="wqk", bufs=16) as wqk,
                    tc.tile_pool(name="psqk", bufs=8, space="PSUM") as psqk,
                ):
                    proj_packed(wqp, xq_sb, qt_sb,
                                bq_sb if with_bias else None, psqk, wqk)
                    proj_packed(wkp, xk_sb, kt_sb,
                                bk_sb if with_bias else None, psqk, wqk)

            # ---- V projection (x stationary, 2048-wide weight stream) ----
            with tc.tile_pool(name="wvo", bufs=10) as wvo:
                with tc.tile_pool(name="psv", bufs=8, space="PSUM") as psv:
                    for r in range(2):
                        pv = [
                            [
                                psv.tile([128, 512], F32, name="pv", tag="pv")
                                for _ in range(4)
                            ]
                            for _ in range(2)
                        ]
                        if with_bias:
                            for tt in range(2):
                                for di in range(4):
                                    bsl = slice(r * 2048 + di * 512,
                                                r * 2048 + (di + 1) * 512)
                                    nc.tensor.matmul(
                                        pv[tt][di][:], lhsT=ones_row[:],
                                        rhs=bv_sb[:, bsl],
                                        start=True, stop=False,
                                    )
                        for kt in range(NK):
                            ch = wvo.tile([128, 2048], BF16, name="cv", tag="cvo")
                            next(queues).dma_start(out=ch[:], in_=wv[r, kt])
                            for tt in range(2):
                                for di in range(4):
                                    nc.tensor.matmul(
                                        pv[tt][di][:],
                                        lhsT=xv_sb[:, kt, tt * 128 : (tt + 1) * 128],
                                        rhs=ch[:, di * 512 : (di + 1) * 512],
                                        start=(kt == 0 and not with_bias),
                                        stop=(kt == NK - 1),
                                    )
                        for tt in range(2):
                            vdst = v0_sb if tt == 0 else v1_sb
                            for di in range(4):
                                dsl = slice(r * 2048 + di * 512,
                                            r * 2048 + (di + 1) * 512)
                                if tt == 0:
                                    nc.scalar.activation(
                                        out=vdst[:, dsl], in_=pv[tt][di][:],
                                        func=Ident,
                                    )
                                else:
                                    nc.vector.tensor_copy(
                                        vdst[:, dsl], pv[tt][di][:]
                                    )

                # ---- attention ----
                # phase A: per head scores^T, exp, col sums, reciprocals
                with (
                    tc.tile_pool(name="psa", bufs=4, space="PSUM") as psa,
                    tc.tile_pool(name="psr", bufs=2, space="PSUM") as psr,
                    tc.tile_pool(name="psb", bufs=1, space="PSUM") as psb,
                ):
                    novt = 16 * amain  # first overflow tile index
                    for h in range(H):
                        eth = et4[h // 4][:, (h % 4) * 2 : (h % 4) * 2 + 2, :]
                        nmm = amain + rem
                        for tt in range(2):
                            pss = psa.tile([128, S], F32, name="pa", tag="pa")
                            tsl = slice(tt * 128, (tt + 1) * 128)
                            i = 0
                            for t in range(amain):
                                mt = h * amain + t
                                nc.tensor.matmul(
                                    pss[:],
                                    lhsT=kt_sb[:, mt, tsl],
                                    rhs=qt_sb[:, mt, :],
                                    start=(i == 0), stop=(i == nmm - 1),
                                )
                                i += 1
                            for ov in range(rem):
                                u = h * rem + ov
                                ovt = novt + u // 3
                                ro = 32 * (u % 3)
                                nc.tensor.matmul(
                                    pss[:],
                                    lhsT=kt_sb[ro : ro + 32, ovt, tsl],
                                    rhs=qt_sb[ro : ro + 32, ovt, :],
                                    start=(i == 0), stop=(i == nmm - 1),
                                )
                                i += 1
                            nc.scalar.activation(
                                out=eth[:, tt, :], in_=pss[:], func=Exp
                            )
                        # column sums of exp (over t = partitions) via matmul
                        prs = psr.tile([1, S], F32, name="pr", tag="pr")
                        nc.tensor.matmul(
                            prs[:], lhsT=ones_col[:], rhs=eth[:, 0, :],
                            start=True, stop=False,
                        )
                        nc.tensor.matmul(
                            prs[:], lhsT=ones_col[:], rhs=eth[:, 1, :],
                            start=False, stop=True,
                        )
                        nc.vector.reciprocal(rinv_f[:, h, :], prs[:])
                        nc.gpsimd.tensor_copy(rinv_b[:, h, :], rinv_f[:, h, :])

                    # phase B: attention @ V; reciprocals are broadcast
                    # across partitions by a rank-1 ones-matmul per 4 heads
                    for q in range(4):
                        for hh in range(2):
                            pb = psb.tile([128, 2 * S], F32, name="pb", tag="pb")
                            h0 = 4 * q + 2 * hh
                            nc.tensor.matmul(
                                pb[:], lhsT=ones_row[:],
                                rhs=rinv_b[:, h0 : h0 + 2, :],
                                start=True, stop=True,
                            )
                            nc.vector.tensor_copy(rbc[:, h0 : h0 + 2, :], pb[:])
                        for h in range(4 * q, 4 * q + 4):
                            for j in range(2):
                                csl = slice(h * HD + j * 128, h * HD + (j + 1) * 128)
                                pu = psa.tile([128, S], F32, name="pc", tag="pa")
                                nc.tensor.matmul(
                                    pu[:], lhsT=v0_sb[:, csl],
                                    rhs=et4[h // 4][:, (h % 4) * 2, :],
                                    start=True, stop=False,
                                )
                                nc.tensor.matmul(
                                    pu[:], lhsT=v1_sb[:, csl],
                                    rhs=et4[h // 4][:, (h % 4) * 2 + 1, :],
                                    start=False, stop=True,
                                )
                                nc.vector.tensor_mul(
                                    at_sb[:, 2 * h + j, :], pu[:], rbc[:, h, :]
                                )

                # ---- output projection (attn^T stationary, Wo streamed) ----
                with tc.tile_pool(name="pso", bufs=8, space="PSUM") as pso:
                    for r in range(2):
                        po = [
                            [
                                pso.tile([128, 512], F32, name="po", tag="po")
                                for _ in range(4)
                            ]
                            for _ in range(2)
                        ]
                        if with_bias:
                            for sh in range(2):
                                for di in range(4):
                                    bsl = slice(r * 2048 + di * 512,
                                                r * 2048 + (di + 1) * 512)
                                    nc.tensor.matmul(
                                        po[sh][di][:], lhsT=ones_row[:],
                                        rhs=bo_sb[:, bsl],
                                        start=True, stop=False,
                                    )
                        for ct in range(NK):
                            ch = wvo.tile([128, 2048], BF16, name="co", tag="cvo")
                            next(queues).dma_start(out=ch[:], in_=wo[r, ct])
                            for sh in range(2):
                                for di in range(4):
                                    nc.tensor.matmul(
                                        po[sh][di][:],
                                        lhsT=at_sb[:, ct, sh * 128 : (sh + 1) * 128],
                                        rhs=ch[:, di * 512 : (di + 1) * 512],
                                        start=(ct == 0 and not with_bias),
                                        stop=(ct == NK - 1),
                                    )
                        for sh in range(2):
                            ot = outp.tile([128, 2048], BF16, name="ot", tag="ot")
                            for di in range(4):
                                osl = slice(di * 512, (di + 1) * 512)
                                if sh == 0:
                                    nc.scalar.activation(
                                        out=ot[:, osl], in_=po[sh][di][:],
                                        func=Ident,
                                    )
                                else:
                                    nc.vector.tensor_copy(
                                        ot[:, osl], po[sh][di][:]
                                    )
                            nc.sync.dma_start(out=out[sh, r], in_=ot[:])

    return nc


_cached = {}


def _get_nc(key=None):
    if key is None:
        key = _cached.get("last_key")
    assert key is not None, "build_in_maps must run before _get_nc"
    if key not in _cached:
        _cached[key] = build_nc(*key)
    _cached["last_key"] = key
    return _cached[key]


def _pack_gidx(mask_b, amain, rem, ntq):
    """Map packed column p -> global W column (h*256 + channel), -1 = pad."""
    surv = np.nonzero(np.asarray(mask_b) != 0)[0]
    M = len(surv)
    P = ntq * 128
    gidx = np.full(P, -1, np.int64)
    main_n = min(M, 128 * amain)
    for h in range(H):
        base = h * amain * 128
        gidx[base : base + main_n] = h * 256 + surv[:main_n]
        for i in range(rem):
            u = h * rem + i
            t = 16 * amain + u // 3
            ro = 32 * (u % 3)
            p0 = t * 128 + ro
            lo = 128 * amain + i * 32
            n = min(max(M - lo, 0), 32)
            if n > 0:
                gidx[p0 : p0 + n] = h * 256 + surv[lo : lo + n]
    return gidx


def _pack_w(W, scale, gidx, ngq):
    wt = np.ascontiguousarray(W.T * scale).astype(np.float32)  # [k, dout]
    P = len(gidx)
    pk = np.zeros((D, P), np.float32)
    valid = gidx >= 0
    pk[:, valid] = wt[:, gidx[valid]]
    pk = pk.astype(NPBF16)
    pk = pk.reshape(16, 2, 128, ngq, 256).transpose(3, 0, 2, 1, 4)
    return np.ascontiguousarray(pk.reshape(ngq, 16, 128, 512))


def _chunks2048(W, scale):
    # W.T with dout split into 2 rounds of 2048: [2, 32, 128, 2048]
    wt = (W.T * scale).astype(NPBF16)  # [k, dout]
    return np.ascontiguousarray(
        wt.reshape(NK, 128, 2, 2048).transpose(2, 0, 1, 3)
    )


def build_in_maps(q, k, v, mask, Wq, bq, Wk, bk, Wv, bv, Wo, bo):
    q = np.asarray(q, dtype=np.float32)
    k = np.asarray(k, dtype=np.float32)
    v = np.asarray(v, dtype=np.float32)
    mask = np.asarray(mask)
    Wq, bq = np.asarray(Wq, np.float32), np.asarray(bq, np.float32)
    Wk, bk = np.asarray(Wk, np.float32), np.asarray(bk, np.float32)
    Wv, bv = np.asarray(Wv, np.float32), np.asarray(bv, np.float32)
    Wo, bo = np.asarray(Wo, np.float32), np.asarray(bo, np.float32)

    Ms = [(mask[b] != 0).sum() for b in range(B)]
    Mhat = max(32, int(-(-max(Ms) // 32) * 32))  # ceil to 32, common
    amain, rem = Mhat // 128, (Mhat % 128) // 32
    ntq = _qk_tiles(amain, rem)
    ngq = ntq // 2
    with_bias = any(np.any(x != 0) for x in (bq, bk, bv, bo))
    key = (amain, rem, with_bias)
    _cached["last_key"] = key

    wv_c = _chunks2048(Wv, 1.0)
    wo_c = _chunks2048(Wo, 1.0 / 64.0)
    if with_bias:
        bvv = np.ascontiguousarray(bv.astype(NPBF16).reshape(1, D))
        bov = np.ascontiguousarray(bo.astype(NPBF16).reshape(1, D))

    in_maps = []
    for b in range(B):
        gidx = _pack_gidx(mask[b], amain, rem, ntq)
        wq_p = _pack_w(Wq, 1.0 / 16.0, gidx, ngq)
        wk_p = _pack_w(Wk, 1.0, gidx, ngq)

        def xt(x):
            # [128 partition, NK k-tile, S] with 16KB contiguous per partition
            t = x[b].T.reshape(NK, 128, S).swapaxes(0, 1)
            return np.ascontiguousarray(t).astype(NPBF16)

        im = dict(xq=xt(q), xk=xt(k), xv=xt(v), wqp=wq_p, wkp=wk_p,
                  wv=wv_c, wo=wo_c)
        if with_bias:
            def pb(bias):
                pkb = np.zeros(ntq * 128, np.float32)
                valid = gidx >= 0
                pkb[valid] = (bias / 16.0 if bias is bq else bias)[gidx[valid] ]
                return np.ascontiguousarray(pkb.reshape(ntq, 128).T)
            im.update(bqp=pb(bq), bkp=pb(bk), bvv=bvv, bov=bov)
        in_maps.append(im)
    return in_maps


def unshard(results):
    outs = []
    for b in range(B):
        blk = np.asarray(results[b]["out"], np.float32)  # [2, 2, 128, 2048]
        o = np.empty((S, D), np.float32)
        for sh in range(2):
            for r in range(2):
                o[sh * 128 : (sh + 1) * 128, r * 2048 : (r + 1) * 2048] = blk[sh, r]
        outs.append(o)
    return np.ascontiguousarray(np.stack(outs))


def kernel(q, k, v, mask, Wq, bq, Wk, bk, Wv, bv, Wo, bo):
    _install_ntff_hook()
    _patch_bir_wait_split()
    in_maps = build_in_maps(q, k, v, mask, Wq, bq, Wk, bk, Wv, bv, Wo, bo)
    nc = _get_nc()

    from concourse.bass_utils import run_bass_kernel_spmd

    res = run_bass_kernel_spmd(nc, in_maps, list(range(N_CORES)))
    return unshard(res.results)
